# revision 54
# baseline (speedup 1.0000x reference)
"""Binarized 3x3 conv (GeneralConv2d) on 8 NeuronCores — fp8 DoubleRow version.

y[b,o,h,w] = mean_abs(w[o]) * sum_{c,kh,kw} sign(w[o,c,kh,kw]) * x[b,c,h+kh-1,w+kw-1]

Data-parallel over batch: 4 images per core on 8 cores; the tiny binarized
weight is replicated.  The conv runs on the tensor engine as fp8e4
DoubleRow matmuls (256-deep contraction per instruction: the two in-channel
chunks ride in the k-tile pair dim).  x is sent as an e4m3 hi/lo pair
(hi = e4m3(x) cast in-DMA, lo = e4m3(bf16(x) - hi)), which keeps the fp8
path at bf16-level accuracy: 18 DoubleRow matmuls accumulate hi and lo taps
into one PSUM bank per 8-row output chunk.

Host-side layout prep (data movement only): x channels are halo-padded to
flat [58*58+2] lines so every load DMA is one contiguous piece per
partition, and the weight is additionally shipped transposed ([ckk, out])
so the sign matrix lands in lhsT layout straight off the DMA — no PE
transposes, freeing all 8 PSUM banks for the conv.

Per-chunk moving operand is one contiguous 464-column window (8 rows x 58);
the 2 halo junk columns per row land in PSUM columns that are never
evicted.  Per-channel mean-abs scaling is applied on PSUM eviction (ACT).
"""

import numpy as np

from contextlib import ExitStack

import concourse.bass as bass
import concourse.mybir as mybir
from concourse import bacc
import concourse.tile as tile

dt = mybir.dt
OUT_C = 256
IN_C = 256
KH = KW = 3
KK = KH * KW           # 9
CKK = IN_C * KK        # 2304
P = 128
CC = IN_C // P         # 2 in-channel chunks (the DoubleRow k-tile pair)
OO = OUT_C // P        # 2 out-channel chunks
QC = CKK // CC         # 1152 columns per (oo,cc) quarter


def _build_conv_nc(imgs: int, H: int, W: int, hchunk: int = 8, psum_bufs: int = 8):
    assert H % hchunk == 0
    nch = H // hchunk
    Hp, Wp = H + 2, W + 2
    FLAT = Hp * Wp         # 3364
    FPAD = FLAT + 2        # +2 so the last tap window stays in-bounds
    MV = hchunk * Wp       # 464-col window; matmuls use 462 (the trailing
                           # halo pair is never read by the last row's taps)
    nc = bacc.Bacc("TRN2", target_bir_lowering=False, debug=False,
                   enable_asserts=False, num_devices=8)
    x = nc.declare_dram_parameter("x", [imgs, IN_C, FPAD], dt.float32,
                                  isOutput=False)
    w = nc.declare_dram_parameter("w", [OUT_C * CKK, 1], dt.float32, isOutput=False)
    w2d = w.rearrange("(o r) one -> o (r one)", r=CKK)   # [256, 2304]
    wt_d = nc.declare_dram_parameter("wt", [OO, CC, P, KK * P], dt.bfloat16,
                                     isOutput=False)
    y = nc.declare_dram_parameter("y", [imgs, OUT_C, H, W], dt.float32, isOutput=True)


    with tile.TileContext(nc) as tc, ExitStack() as ctx:
        wprep = ctx.enter_context(tc.tile_pool(name="wprep", bufs=1))
        w_sb = wprep.tile([P, OO, CKK], dt.bfloat16)
        wts_sb = wprep.tile([P, OO, CC, KK, P], dt.bfloat16)
        bsgn_sb = wprep.tile([P, OO, CC, KK, P], dt.bfloat16)
        wt8 = wprep.tile([P, OO, CC, KK, P], dt.float8e4)
        scale_sb = wprep.tile([P, OO], dt.float32)
        neg1 = wprep.tile([P, 1], dt.float32)
        nc.vector.memset(neg1, -1.0)
        scratch = wprep.tile([P, 512], dt.bfloat16)
        nc.vector.memset(scratch, 0.0)

        xhp = ctx.enter_context(tc.tile_pool(name="xhi", bufs=imgs))
        xlp = ctx.enter_context(tc.tile_pool(name="xlo", bufs=imgs))
        xbp = ctx.enter_context(tc.tile_pool(name="xbf", bufs=2))

        hi_t = {}
        lo_t = {}
        xb_t = {}

        def load_x(img, defer_subs=False):
            hi = xhp.tile([P, CC, FPAD], dt.float8e4, name=f"hi{img}", tag="hi")
            lo = xlp.tile([P, CC, FPAD], dt.float8e4, name=f"lo{img}", tag="lo")
            xb = xbp.tile([P, CC, FPAD], dt.bfloat16, name=f"xb{img}", tag="xb")
            h2 = FPAD // 2

            def pieces(t, half, only_cc=None):
                s = slice(half * h2, FPAD if half else h2)
                for cc in range(CC):
                    if only_cc is not None and cc != only_cc:
                        continue
                    # fp32->fp8 / fp32->bf16 casts happen in-DMA (gpsimd).
                    nc.gpsimd.dma_start(out=t[:, cc, s],
                                        in_=x[img, cc * P:(cc + 1) * P, s])

            # First halves of hi (pass A) and xb (residual) land first; on
            # img0 the sign quarters and the scale source ride between so the
            # wire alternates weight-vs-x pieces in dependency order.
            if img == 0:
                dma_w_signs(0)
                dma_w_signs(1)
            pieces(hi, 0)
            pieces(xb, 0)
            if img == 0:
                dma_w_scale()
            pieces(hi, 1)
            pieces(xb, 1)
            hi_t[img], lo_t[img], xb_t[img] = hi, lo, xb
            if not defer_subs:
                sub_x(img)

        def sub_x_h1(img):
            # lo = e4m3(bf16(x) - hi), full flat rows (halo stays zero).
            # First halves on DVE, in quarter pieces so the scheduler can
            # slot the small weight-sign ops between them.
            hi, lo, xb = hi_t[img], lo_t[img], xb_t[img]
            h2 = FPAD // 2
            q = h2 // 2
            for cc in range(CC):
                nc.vector.tensor_sub(lo[:, cc, 0:q], xb[:, cc, 0:q],
                                     hi[:, cc, 0:q])
                nc.vector.tensor_sub(lo[:, cc, q:h2], xb[:, cc, q:h2],
                                     hi[:, cc, q:h2])

        def sub_x_h2(img, pool=True):
            # Second halves on Pool (DVE for img0, whose Pool is still busy
            # issuing the startup DMAs).
            hi, lo, xb = hi_t[img], lo_t[img], xb_t[img]
            h2 = FPAD // 2
            e2 = nc.gpsimd if pool else nc.vector
            e2.tensor_sub(lo[:, 0, h2:FPAD], xb[:, 0, h2:FPAD],
                          hi[:, 0, h2:FPAD])
            e2.tensor_sub(lo[:, 1, h2:FPAD], xb[:, 1, h2:FPAD],
                          hi[:, 1, h2:FPAD])

        def sub_x(img, h2_pool=True):
            sub_x_h1(img)
            sub_x_h2(img, pool=h2_pool)

        def dma_w_signs(oo):
            # Sign-source (transposed, host-cast bf16 — sign-exact) quarters
            # gate the first matmuls; non-cast DMAs ride the HWDGE queue in
            # parallel with Pool's x issues.
            for cc in range(CC):
                nc.sync.dma_start(
                    out=wts_sb[:, oo, cc].rearrange("p kk o -> p (kk o)"),
                    in_=wt_d[oo, cc])

        def dma_w_scale():
            for oo in range(OO):
                nc.gpsimd.dma_start(out=w_sb[:, oo, :],
                                    in_=w2d[oo * P:(oo + 1) * P, :])

        def sgn_w(oo, cc):
            # b = (w >= 0) in {0,1} on DVE, then 2b-1 -> fp8 on ACT.
            nc.vector.tensor_scalar(
                out=bsgn_sb[:, oo, cc],
                in0=wts_sb[:, oo, cc],
                scalar1=0.0, scalar2=1.0,
                op0=mybir.AluOpType.is_ge, op1=mybir.AluOpType.mult)
            nc.scalar.activation(
                out=wt8[:, oo, cc], in_=bsgn_sb[:, oo, cc],
                func=mybir.ActivationFunctionType.Identity,
                bias=neg1[:, 0:1], scale=2.0)

        def reduce_scale(oo):
            # Per-out-channel scale column (runs on DVE behind the conv).
            nc.vector.tensor_reduce(
                out=scale_sb[:, oo:oo + 1], in_=w_sb[:, oo, :],
                axis=mybir.AxisListType.X,
                op=mybir.AluOpType.add, apply_absolute_value=True)
            nc.vector.tensor_scalar_mul(
                scale_sb[:, oo:oo + 1], scale_sb[:, oo:oo + 1], 1.0 / CKK)

        pp = ctx.enter_context(tc.tile_pool(name="psum", bufs=psum_bufs, space="PSUM"))
        op = ctx.enter_context(tc.tile_pool(name="ostage", bufs=6))

        def warmup_pe(n=6):
            # Dummy matmuls on a zeroed scratch tile keep the PE busy through
            # its p-state ramp while the first loads/sign prep are in flight,
            # so the real conv matmuls start at full clock.
            wps = pp.tile([P, 512], dt.float32, name="warm_ps", tag="ps")
            for _ in range(n):
                nc.tensor.matmul(wps, lhsT=scratch[:, 0:P], rhs=scratch,
                                 start=True, stop=True)

        def mm(ps, src_t, oo, ih, k, n):
            ki, kj = divmod(k, KW)
            fs = (ih * hchunk + ki) * Wp + kj
            mv = MV - 2
            nc.tensor.matmul(ps[:, 0:mv], lhsT=wt8[:, oo, :, k, :],
                             rhs=src_t[:, :, fs:fs + mv],
                             start=(n == 0), stop=(n == 2 * KK - 1),
                             perf_mode=mybir.MatmulPerfMode.DoubleRow)

        def conv_a(img, oo, tiles):
            # Pass A: the 9 hi-taps for the group's tiles (start accumulation).
            group = {}
            for ih in tiles:
                ps = pp.tile([P, MV], dt.float32,
                             name=f"ps_{img}_{oo}_{ih}", tag="ps")
                group[ih] = ps
                for k in range(KK):
                    mm(ps, hi_t[img], oo, ih, k, n=k)
            return group

        ydma_flip = [0]

        def conv_b(img, oo, group, final=False):
            # Pass B: the 9 lo-taps, then scale + store.  The very last chunk
            # evicts and stores in two half-height pieces so the end-of-kernel
            # drain only waits on half a store chain.
            items = list(group.items())
            for gi, (ih, ps) in enumerate(items):
                for k in range(KK):
                    mm(ps, lo_t[img], oo, ih, k, n=KK + k)
                psv = ps.rearrange("p (h w) -> p h w", w=Wp)
                halves = 2 if (final and gi == len(items) - 1) else 1
                hc2 = hchunk // halves
                for hh in range(halves):
                    st = op.tile([P, hc2, W], dt.float32,
                                 name=f"st_{img}_{oo}_{ih}_{hh}",
                                 tag="st" if halves == 1 else "st2")
                    nc.scalar.mul(st, psv[:, hh * hc2:(hh + 1) * hc2, 0:W],
                                  scale_sb[:, oo:oo + 1])
                    # Alternate store DMAs between the two HWDGE queues.
                    eng = nc.sync if ydma_flip[0] % 2 == 0 else nc.scalar
                    ydma_flip[0] += 1
                    eng.dma_start(
                        out=y[img, oo * P:(oo + 1) * P,
                              ih * hchunk + hh * hc2:
                              ih * hchunk + (hh + 1) * hc2, :],
                        in_=st)

        def conv(img, mid=None, last=False):
            # 3+3 leading hi-groups across both oo halves, then 4+4 trailing
            # groups; max 8 PSUM banks in flight, and the lo-residual latency
            # hides behind the leading hi-only matmul burst.  The last image
            # splits its trailing oo1 groups in two so the final stores are
            # spread instead of bunched behind the very last matmul.
            g1 = min(3, nch)
            a00 = conv_a(img, 0, list(range(g1)))
            a10 = conv_a(img, 1, list(range(g1)))
            conv_b(img, 0, a00)
            a01 = conv_a(img, 0, list(range(g1, nch)))
            conv_b(img, 1, a10)
            if mid is not None:
                mid()
            if not last:
                a11 = conv_a(img, 1, list(range(g1, nch)))
                conv_b(img, 0, a01)
                conv_b(img, 1, a11)
            else:
                m = nch - 1
                a11a = conv_a(img, 1, list(range(g1, m)))
                conv_b(img, 0, a01)
                a11b = conv_a(img, 1, list(range(m, nch)))
                conv_b(img, 1, a11a)
                conv_b(img, 1, a11b)

        # Emission order seeds per-engine program order: img0's loads are
        # issued first (Pool/SWDGE serializes descriptor generation), all
        # four weight quarters are sign-prepped before img0's residual subs
        # so the DVE work that gates the tensor engine never queues behind
        # a DMA-waiting sub.
        load_x(0, defer_subs=True)
        with tc.high_priority():
            warmup_pe()
        with tc.high_priority():
            for oo in range(OO):
                for cc in range(CC):
                    sgn_w(oo, cc)
        sub_x_h1(0)
        reduce_scale(0)
        reduce_scale(1)
        sub_x_h2(0, pool=False)
        conv(0, mid=(lambda: load_x(1)) if imgs > 1 else None, last=imgs == 1)
        for img in range(1, imgs):
            conv(img, mid=(lambda i=img: load_x(i + 1)) if img + 1 < imgs else None,
                 last=img == imgs - 1)
    nc.compile()
    return nc


BATCH, H, W = 32, 56, 56
N_CORES = 8
IMGS = BATCH // N_CORES
_NC_CACHE = {}


def _get_nc():
    key = (IMGS, H, W)
    if key not in _NC_CACHE:
        _NC_CACHE[key] = _build_conv_nc(IMGS, H, W, hchunk=8, psum_bufs=8)
    return _NC_CACHE[key]


def kernel(**inputs) -> np.ndarray:
    from concourse.bass_utils import run_bass_kernel_spmd

    x = np.asarray(inputs["x"], dtype=np.float32)
    weight = np.ascontiguousarray(np.asarray(inputs["weight"], dtype=np.float32))
    assert x.shape == (BATCH, IN_C, H, W), x.shape
    assert weight.shape == (OUT_C * CKK, 1), weight.shape

    # Host-side layout prep (pure data movement): halo-pad each image channel
    # to flat [58*58(+2)] lines, and ship the weight both as-is (scale
    # reduction) and transposed (sign matrix in lhsT layout).
    Hp, Wp = H + 2, W + 2
    xpad = np.zeros((BATCH, IN_C, Hp * Wp + 2), dtype=np.float32)
    xpad[:, :, :Hp * Wp].reshape(BATCH, IN_C, Hp, Wp)[:, :, 1:H + 1, 1:W + 1] = x
    import ml_dtypes
    wT = np.ascontiguousarray(
        weight.reshape(OO, P, CC, P, KK).transpose(0, 2, 3, 4, 1)
        .reshape(OO, CC, P, KK * P).astype(ml_dtypes.bfloat16))

    nc = _get_nc()
    in_maps = [
        {"x": xpad[c * IMGS:(c + 1) * IMGS], "w": weight, "wt": wT}
        for c in range(N_CORES)
    ]
    res = run_bass_kernel_spmd(nc, in_maps, core_ids=list(range(N_CORES)))
    return np.concatenate([res.results[c]["y"] for c in range(N_CORES)], axis=0)
